# revision 1
# baseline (speedup 1.0000x reference)
"""BiGRU encoder kernel for 8 Trainium2 NeuronCores.

Strategy:
  - Reformulate the per-sample ragged windows as masked GRUs over FIXED
    position ranges: forward runs positions 0..7 ascending, backward runs
    positions 14..7 descending.  A sample with forward length lf only starts
    updating at position 8-lf; before that its hidden state must stay 0.
    That is enforced exactly by adding +BIG to the z-gate pre-activation for
    pre-start steps (z==1.0 => h' = n + z*(h-n) = n + (0-n) = 0 exactly).
  - Sort samples by window_len, deal them round-robin to the 8 cores (data
    parallel, near-identical length distribution per core).  Per core, two
    batch tiles of 512 samples; each GRU step runs only on the suffix of
    samples that are long enough to need it (suffix clamped to >=256 so
    float32r matmuls stay at full rate; over-included samples are exact via
    the z-mask and h-prefix memsets).
  - Everything on-device is computed in transposed (feature-major) layout:
    features on SBUF partitions, samples on the free dim, so the recurrence
    needs no runtime transposes.  Weights are transposed host-side.
  - Matmuls run as float32r (full-rate fp32 mode of the PE array).
"""

import os
from contextlib import ExitStack

import numpy as np

import concourse.bacc as bacc
import concourse.tile as tile
from concourse import mybir
from concourse.bass_utils import run_bass_kernel_spmd
from concourse.masks import make_identity

NCORES = 8
B, T, D, H = 8192, 15, 512, 512
G = 3 * H  # gate rows (r, z, n)
BIG = 40.0
S = 512  # samples per batch tile
F32 = mybir.dt.float32
DT_MM = mybir.dt.float32 if os.environ.get("GRU_DT") == "f32" else mybir.dt.float32r
H_ENGINE = os.environ.get("GRU_HUPD", "vector")  # engine for h-update chain

ACT = mybir.ActivationFunctionType
ALU = mybir.AluOpType

_PROGRAM_CACHE = {}
LAST_RESULT = None


def _build_program(sched):
    """sched: per tile, (f_steps, b_steps); each step = (width, masked)."""
    ntiles = len(sched)
    Bc = S * ntiles
    nc = bacc.Bacc("TRN2", target_bir_lowering=False, debug=False,
                   num_devices=NCORES)

    xT_d = nc.dram_tensor("xT", [T, D, Bc], DT_MM, kind="ExternalInput")
    wf_d = nc.dram_tensor("wf", [D + H, G], DT_MM, kind="ExternalInput")
    wb_d = nc.dram_tensor("wb", [D + H, G], DT_MM, kind="ExternalInput")
    w1_d = nc.dram_tensor("w1", [2 * H, H], DT_MM, kind="ExternalInput")
    w2_d = nc.dram_tensor("w2", [H, H], DT_MM, kind="ExternalInput")
    bias_d = nc.dram_tensor("bias", [40, 128], F32, kind="ExternalInput")
    mf_d = nc.dram_tensor("maskzf", [8, Bc], F32, kind="ExternalInput")
    mb_d = nc.dram_tensor("maskzb", [8, Bc], F32, kind="ExternalInput")
    y_d = nc.dram_tensor("y", [Bc, H], F32, kind="ExternalOutput")

    with tile.TileContext(nc) as tc, ExitStack() as ctx:
        const = ctx.enter_context(tc.tile_pool(name="const", bufs=1))
        wpool = ctx.enter_context(tc.tile_pool(name="w", bufs=2))
        xpool = ctx.enter_context(tc.tile_pool(name="x", bufs=2))
        hpool = ctx.enter_context(tc.tile_pool(name="h", bufs=2))
        hfin = ctx.enter_context(tc.tile_pool(name="hfin", bufs=4))
        gpool = ctx.enter_context(tc.tile_pool(name="g", bufs=5))
        mpool = ctx.enter_context(tc.tile_pool(name="m", bufs=1))
        opool = ctx.enter_context(tc.tile_pool(name="o", bufs=4))
        rzps = ctx.enter_context(tc.tile_pool(name="rz", bufs=4, space="PSUM"))
        xpps = ctx.enter_context(tc.tile_pool(name="xp", bufs=2, space="PSUM"))
        ghps = ctx.enter_context(tc.tile_pool(name="gh", bufs=2, space="PSUM"))

        # Weights [128, kchunk, gate-cols]; kchunks 0-3 input dims, 4-7 hidden
        # dims.  wf/wb/w1 time-share a 2-slot pool (one tag); per-kchunk DMAs
        # so the first matmuls start as soon as chunk 0 lands.
        def load_w(dram, kchunks, cols, name, pool=None, sync_chunks=()):
            t_ = (pool or wpool).tile([128, kchunks, cols], DT_MM,
                                      tag="w" if pool is None else "const",
                                      name=name)
            src = dram.rearrange("(c k) g -> k c g", k=128)
            for c in range(kchunks):
                eng = nc.sync if c in sync_chunks else nc.scalar
                eng.dma_start(t_[:, c, :], src[:, c, :])
            return t_

        wf = load_w(wf_d, 8, G, "wf", sync_chunks=(0, 1, 2, 3))
        wb = load_w(wb_d, 8, G, "wb")
        w2 = load_w(w2_d, 4, H, "w2", pool=const)
        bt = const.tile([128, 40], F32)
        nc.gpsimd.dma_start(bt[:], bias_d.rearrange("n p -> p n"))
        ident = const.tile([128, 128], F32)
        make_identity(nc, ident[:])

        heng = nc.gpsimd if H_ENGINE == "gpsimd" else nc.vector

        def emit_dir(s0, steps, w, mask_d, bb, pos_fn):
            """One GRU direction over one batch tile; returns final h tile."""
            nsteps = len(steps)
            h_prev = None
            for j, (width, masked) in enumerate(steps):
                first = j == 0
                p_abs = pos_fn(j)
                so = S - width  # suffix offset within the tile
                a0 = s0 + so
                xt = xpool.tile([128, 4, S], DT_MM, tag="x", name="xt")
                nc.sync.dma_start(
                    xt[:, :, :width],
                    xT_d[p_abs].rearrange("(c k) s -> k c s", k=128)[:, :, a0:s0 + S],
                )
                mt = None
                if masked:
                    mt = mpool.tile([128, S], F32, tag="m", name="mt")
                    nc.gpsimd.dma_start(
                        mt[:, :width],
                        mask_d[8 - nsteps + j, a0:s0 + S].partition_broadcast(128),
                    )
                h_next = (hfin if j == nsteps - 1 else hpool).tile(
                    [128, 4, S], DT_MM, tag="hf" if j == nsteps - 1 else "h",
                    name="h")
                if j + 1 < nsteps:
                    nso = S - steps[j + 1][0]  # next step's suffix offset
                    if nso < so:
                        nc.gpsimd.memset(h_next[:, :, nso:so].bitcast(F32), 0.0)

                rps, zps, xpns, ghns = [], [], [], []
                for i in range(4):
                    # separate PSUM tiles per accumulation group: start=True
                    # clears the whole bank, so groups must not share one
                    r_ps = rzps.tile([128, width], F32, tag="rz", name=f"rps{i}")
                    z_ps = rzps.tile([128, width], F32, tag="rz", name=f"zps{i}")
                    xpn = xpps.tile([128, width], F32, tag="xp", name=f"xpn{i}")
                    rps.append(r_ps)
                    zps.append(z_ps)
                    xpns.append(xpn)
                    for k in range(4):
                        st = k == 0
                        sp_rz = first and k == 3
                        xk = xt[:, k, :width]
                        nc.tensor.matmul(r_ps[:], w[:, k, i * 128:(i + 1) * 128],
                                         xk, start=st, stop=sp_rz)
                        nc.tensor.matmul(z_ps[:],
                                         w[:, k, H + i * 128:H + (i + 1) * 128],
                                         xk, start=st, stop=sp_rz)
                        nc.tensor.matmul(xpn[:],
                                         w[:, k, 2 * H + i * 128:2 * H + (i + 1) * 128],
                                         xk, start=st, stop=k == 3)
                if not first:
                    for i in range(4):
                        ghn = ghps.tile([128, width], F32, tag="gh", name=f"ghn{i}")
                        ghns.append(ghn)
                        for k in range(4):
                            hk = h_prev[:, k, so:]
                            nc.tensor.matmul(rps[i][:],
                                             w[:, 4 + k, i * 128:(i + 1) * 128],
                                             hk, start=False, stop=k == 3)
                            nc.tensor.matmul(zps[i][:],
                                             w[:, 4 + k, H + i * 128:H + (i + 1) * 128],
                                             hk, start=False, stop=k == 3)
                            nc.tensor.matmul(ghn[:],
                                             w[:, 4 + k, 2 * H + i * 128:2 * H + (i + 1) * 128],
                                             hk, start=k == 0, stop=k == 3)

                for i in range(4):
                    xpn = xpns[i]
                    r = gpool.tile([128, width], F32, tag="g", name="r")
                    nc.scalar.activation(r[:], rps[i][:], ACT.Sigmoid,
                                         bias=bt[:, bb + i:bb + i + 1])
                    if masked:
                        zin = gpool.tile([128, width], F32, tag="g", name="zin")
                        nc.vector.tensor_add(zin[:], zps[i][:], mt[:, :width])
                        zsrc = zin[:]
                    else:
                        zsrc = zps[i][:]
                    z = gpool.tile([128, width], F32, tag="g", name="z")
                    nc.scalar.activation(z[:], zsrc, ACT.Sigmoid,
                                         bias=bt[:, bb + 4 + i:bb + 5 + i])
                    tt = gpool.tile([128, width], F32, tag="g", name="tt")
                    if first:
                        nc.vector.tensor_scalar_mul(tt[:], r[:],
                                                    bt[:, bb + 8 + i:bb + 9 + i])
                    else:
                        nc.vector.scalar_tensor_tensor(
                            tt[:], ghns[i][:], bt[:, bb + 8 + i:bb + 9 + i], r[:],
                            op0=ALU.add, op1=ALU.mult)
                    ss = gpool.tile([128, width], F32, tag="g", name="ss")
                    nc.vector.tensor_add(ss[:], tt[:], xpn[:])
                    n = gpool.tile([128, width], F32, tag="g", name="n")
                    nc.scalar.activation(n[:], ss[:], ACT.Tanh,
                                         bias=bt[:, bb + 12 + i:bb + 13 + i])
                    ho = h_next[:, i, so:]
                    if first:
                        e = gpool.tile([128, width], F32, tag="g", name="e")
                        heng.tensor_mul(e[:], z[:], n[:])
                        heng.tensor_sub(ho, n[:], e[:])
                    else:
                        dd = gpool.tile([128, width], F32, tag="g", name="dd")
                        heng.tensor_sub(dd[:], h_prev[:, i, so:], n[:])
                        e = gpool.tile([128, width], F32, tag="g", name="e")
                        heng.tensor_mul(e[:], z[:], dd[:])
                        heng.tensor_add(ho, n[:], e[:])
                h_prev = h_next
            return h_prev

        hfs = []
        for t in range(ntiles):
            nf = len(sched[t][0])
            hfs.append(emit_dir(t * S, sched[t][0], wf, mf_d, 0,
                                lambda j, nf=nf: 8 - nf + j))
        w1 = load_w(w1_d, 8, H, "w1")

        def emit_mlp(t, hf, hb):
            hid = []
            for i in range(4):
                ps = xpps.tile([128, S], F32, tag="xp", name="mps")
                for k in range(8):
                    src = hf if k < 4 else hb
                    nc.tensor.matmul(ps[:], w1[:, k, i * 128:(i + 1) * 128],
                                     src[:, k % 4, :], start=k == 0, stop=k == 7)
                h32 = gpool.tile([128, S], F32, tag="g", name="h32")
                nc.scalar.activation(h32[:], ps[:], ACT.Relu,
                                     bias=bt[:, 32 + i:33 + i])
                hr = gpool.tile([128, S], DT_MM, tag="g", name="hr")
                nc.vector.tensor_copy(hr[:], h32[:])
                hid.append(hr)
            onats = []
            for gidx in range(S // 128):
                onat = opool.tile([128, H], F32, tag="o", name=f"onat{gidx}")
                onats.append(onat)
            for i in range(4):
                ps = xpps.tile([128, S], F32, tag="xp", name="ops")
                for k in range(4):
                    nc.tensor.matmul(ps[:], w2[:, k, i * 128:(i + 1) * 128],
                                     hid[k][:], start=k == 0, stop=k == 3)
                o32 = gpool.tile([128, S], F32, tag="g", name="o32")
                nc.vector.tensor_scalar_add(o32[:], ps[:], bt[:, 36 + i:37 + i])
                for gidx in range(S // 128):
                    tp = ghps.tile([128, 128], F32, tag="gh", name="tp")
                    nc.tensor.transpose(tp[:], o32[:, gidx * 128:(gidx + 1) * 128],
                                        ident[:])
                    nc.vector.tensor_copy(onats[gidx][:, i * 128:(i + 1) * 128],
                                          tp[:])
            for gidx in range(S // 128):
                r0 = t * S + gidx * 128
                nc.sync.dma_start(y_d[r0:r0 + 128, :], onats[gidx][:])

        for t in range(ntiles):
            nb = len(sched[t][1])
            hb = emit_dir(t * S, sched[t][1], wb, mb_d, 16,
                          lambda j, nb=nb: 6 + nb - j)
            emit_mlp(t, hfs[t], hb)

    nc.compile()
    return nc


def kernel(padded_window, window_len, Wih_f, Whh_f, bih_f, bhh_f,
           Wih_b, Whh_b, bih_b, bhh_b, W1, b1, W2, b2):
    wl = np.asarray(window_len)
    lf = (wl - 1) // 2 + 1
    lb = wl // 2 + 1
    order = np.argsort(wl, kind="stable")

    Bc = B // NCORES
    ntiles = Bc // S
    # per-core sorted lengths: row k = per-core rank k, column = core
    lf_pc = lf[order].reshape(-1, NCORES)
    lb_pc = lb[order].reshape(-1, NCORES)

    def dir_steps(lens_pc, t):
        seg = lens_pc[t * S:(t + 1) * S]  # [S, NCORES]
        n = int(seg.max())
        steps = []
        for j in range(n):
            need = n - j
            cnt = (seg >= need).sum(axis=0)  # samples needing this step, per core
            w = int(min(S, max(256, -(-int(cnt.max()) // 64) * 64)))
            masked = bool(cnt.min() < w)
            steps.append((w, masked))
        return tuple(steps)

    sched = tuple((dir_steps(lf_pc, t), dir_steps(lb_pc, t))
                  for t in range(ntiles))

    if sched not in _PROGRAM_CACHE:
        _PROGRAM_CACHE[sched] = _build_program(sched)
    nc = _PROGRAM_CACHE[sched]

    f32 = np.float32
    wf = np.concatenate([Wih_f.T, Whh_f.T], 0).astype(f32)
    wb = np.concatenate([Wih_b.T, Whh_b.T], 0).astype(f32)
    w1 = np.ascontiguousarray(W1.T, dtype=f32)
    w2 = np.ascontiguousarray(W2.T, dtype=f32)

    def chunks(v):  # [512] -> [4, 128]
        return np.asarray(v, f32).reshape(4, 128)

    bias = np.concatenate([
        chunks((bih_f + bhh_f)[:H]), chunks((bih_f + bhh_f)[H:2 * H]),
        chunks(bhh_f[2 * H:]), chunks(bih_f[2 * H:]),
        chunks((bih_b + bhh_b)[:H]), chunks((bih_b + bhh_b)[H:2 * H]),
        chunks(bhh_b[2 * H:]), chunks(bih_b[2 * H:]),
        chunks(b1), chunks(b2),
    ], 0)  # [40, 128]

    pw = np.asarray(padded_window, f32)
    in_maps = []
    p8 = np.arange(8)
    for c in range(NCORES):
        idx = order[c::NCORES]
        xT = np.ascontiguousarray(pw[idx].transpose(1, 2, 0))  # [15, 512, Bc]
        mzf = (BIG * (p8[:, None] < (8 - lf[idx])[None, :])).astype(f32)
        mzb = (BIG * (p8[:, None] < (8 - lb[idx])[None, :])).astype(f32)
        in_maps.append({
            "xT": xT, "wf": wf, "wb": wb, "w1": w1, "w2": w2,
            "bias": bias, "maskzf": mzf, "maskzb": mzb,
        })

    trace = bool(os.environ.get("GRU_TRACE"))
    kw = {}
    if os.environ.get("GRU_TMPDIR"):
        kw["tmpdir"] = os.environ["GRU_TMPDIR"]
    res = run_bass_kernel_spmd(nc, in_maps, core_ids=list(range(NCORES)),
                               trace=trace, **kw)
    global LAST_RESULT
    LAST_RESULT = res
    out = np.empty((B, H), f32)
    for c in range(NCORES):
        out[order[c::NCORES]] = res.results[c]["y"]
    return out



# revision 2
# speedup vs baseline: 1.6984x; 1.6984x over previous
"""BiGRU encoder kernel for 8 Trainium2 NeuronCores.

Strategy (v2 — mixed precision bf16/fp8):
  - Masked GRUs over fixed position ranges (see v1): forward runs positions
    ascending into the center, backward descending; a sample of length l only
    starts updating z-steps from its start; pre-start samples are forced to
    z==1 (h'==h==0 exactly) by adding +SCALE*BIG to the z-gate preactivation.
  - Sort samples by window_len, deal round-robin to 8 cores; per core two
    batch tiles of 512 sorted samples; per GRU step only the suffix of
    samples that needs the step is processed (width gran 32).
  - Precision: z,n input projections in bf16 (accuracy-critical), r input
    projection and ALL hidden projections in fp8e4m3 with DoubleRow matmuls
    (2 k-chunks per instruction) — GRU recurrence is contractive, so fp8
    noise on the hidden path washes out (measured 1.0e-2 vs 2e-2 budget).
    All weights pre-scaled x16 host-side (fp8 subnormal floor), undone by
    the activation `scale=1/16`.
  - z-mask applied as a rank-1 matmul (stationary [1,128]=640, moving 0/1
    row) accumulated into the z PSUM group — no DVE mask add.
  - n-gate: tt=(ghn+16*bhh_n)*r on DVE (bf16 out), then an identity matmul
    accumulates tt into the xn PSUM group — no separate ss add.
  - h carried in bf16; cast to fp8 (scalar engine) for the next hidden
    matmul.  h-update chain (dd,e,ho) runs merged over 2-kchunk halves in
    bf16 (DVE 2x mode), halves staggered so the next step's first DoubleRow
    pair can start after half 0.
  - The two batch tiles are interleaved step-by-step, and each step is
    emitted in two phases (A: DMAs+projection matmuls+r/z ACTs+tt, B:
    id-matmul+n ACT+h update+cast) so the PE always has the other tile's
    matmul stream to chew while one tile's gate chain drains.
"""

import os
from contextlib import ExitStack

import numpy as np
import ml_dtypes

import concourse.bacc as bacc
import concourse.tile as tile
from concourse import mybir
from concourse.bass_utils import run_bass_kernel_spmd
from concourse.masks import make_identity

NCORES = 8
B, T, D, H = 8192, 15, 512, 512
G = 3 * H
BIG = 40.0
SCALE = 16.0
S = 512  # samples per batch tile
F32 = mybir.dt.float32
BF16 = mybir.dt.bfloat16
F8 = mybir.dt.float8e4
DR_MIN = int(os.environ.get("GRU_DRMIN", "320"))  # min width for DoubleRow

ACT = mybir.ActivationFunctionType
ALU = mybir.AluOpType
DRM = mybir.MatmulPerfMode.DoubleRow

_PROGRAM_CACHE = {}
LAST_RESULT = None


def _build_program(sched):
    """sched: per tile, (f_steps, b_steps); each step = (width, masked)."""
    ntiles = len(sched)
    Bc = S * ntiles
    nc = bacc.Bacc("TRN2", target_bir_lowering=False, debug=False,
                   num_devices=NCORES)

    xb_d = nc.dram_tensor("xb", [T, D, Bc], BF16, kind="ExternalInput")
    x8_d = nc.dram_tensor("x8", [T, D, Bc], F8, kind="ExternalInput")
    wzn_d = {'f': nc.dram_tensor("wznf", [D, 2 * H], BF16, kind="ExternalInput"),
             'b': nc.dram_tensor("wznb", [D, 2 * H], BF16, kind="ExternalInput")}
    wr8_d = {'f': nc.dram_tensor("wr8f", [D, H], F8, kind="ExternalInput"),
             'b': nc.dram_tensor("wr8b", [D, H], F8, kind="ExternalInput")}
    wh8_d = {'f': nc.dram_tensor("wh8f", [H, G], F8, kind="ExternalInput"),
             'b': nc.dram_tensor("wh8b", [H, G], F8, kind="ExternalInput")}
    w1_d = nc.dram_tensor("w1", [2 * H, H], BF16, kind="ExternalInput")
    w2_d = nc.dram_tensor("w2", [H, H], BF16, kind="ExternalInput")
    bias_d = nc.dram_tensor("bias", [40, 128], F32, kind="ExternalInput")
    m_d = {'f': nc.dram_tensor("maskzf", [1, 8 * Bc], BF16, kind="ExternalInput"),
           'b': nc.dram_tensor("maskzb", [1, 8 * Bc], BF16, kind="ExternalInput")}
    y_d = nc.dram_tensor("y", [Bc, H], F32, kind="ExternalOutput")

    with tile.TileContext(nc) as tc, ExitStack() as ctx:
        const = ctx.enter_context(tc.tile_pool(name="const", bufs=1))
        xpool = ctx.enter_context(tc.tile_pool(name="x", bufs=3))
        hpool = ctx.enter_context(tc.tile_pool(name="h", bufs=5))
        h8pool = ctx.enter_context(tc.tile_pool(name="h8", bufs=3))
        gpool = ctx.enter_context(tc.tile_pool(name="g", bufs=10))
        opool = ctx.enter_context(tc.tile_pool(name="o", bufs=4))
        rzps = ctx.enter_context(tc.tile_pool(name="rz", bufs=4, space="PSUM"))
        xpps = ctx.enter_context(tc.tile_pool(name="xp", bufs=2, space="PSUM"))
        ghps = ctx.enter_context(tc.tile_pool(name="gh", bufs=2, space="PSUM"))

        def load_w(dram, kchunks, cols, dt, name, eng):
            t_ = const.tile([128, kchunks, cols], dt, name=name)
            src = dram.rearrange("(c k) g -> k c g", k=128)
            for c in range(kchunks):
                eng.dma_start(t_[:, c, :], src[:, c, :])
            return t_

        wzn = {d: load_w(wzn_d[d], 4, 2 * H, BF16, f"wzn{d}", nc.scalar)
               for d in 'fb'}
        wr8 = {d: load_w(wr8_d[d], 4, H, F8, f"wr8{d}", nc.scalar)
               for d in 'fb'}
        wh8 = {d: load_w(wh8_d[d], 4, G, F8, f"wh8{d}", nc.scalar)
               for d in 'fb'}
        w1 = load_w(w1_d, 8, H, BF16, "w1", nc.gpsimd)
        w2 = load_w(w2_d, 4, H, BF16, "w2", nc.gpsimd)
        bt = const.tile([128, 40], F32)
        nc.gpsimd.dma_start(bt[:], bias_d.rearrange("n p -> p n"))
        mrows = {}
        for d in 'fb':
            mt = const.tile([1, 8 * Bc], BF16, name=f"mrows{d}")
            nc.gpsimd.dma_start(mt[:], m_d[d][:])
            mrows[d] = mt
        ident = const.tile([128, 128], BF16)
        make_identity(nc, ident[:])
        m640 = const.tile([1, 128], BF16)
        nc.vector.memset(m640[:], SCALE * BIG)

        class St:  # per (dir, tile) recurrence state
            h = None
            h8 = None
            so = None

        def phase_a(d, t, steps, j, st):
            """DMAs, projection matmuls, mask, r/z ACTs, tt."""
            w, masked = steps[j]
            nsteps = len(steps)
            first = j == 0
            so = S - w
            s0 = t * S
            a0 = s0 + so
            bb = 0 if d == 'f' else 16
            pos = (8 - nsteps + j) if d == 'f' else (6 + nsteps - j)
            use_dr = w >= DR_MIN

            xt = xpool.tile([128, 4, S], BF16, tag="x", name="xt")
            nc.sync.dma_start(
                xt[:, :, :w],
                xb_d[pos].rearrange("(c k) s -> k c s", k=128)[:, :, a0:s0 + S])
            x8 = xpool.tile([128, 4, S], F8, tag="x8", name="x8t")
            nc.sync.dma_start(
                x8[:, :, :w],
                x8_d[pos].rearrange("(c k) s -> k c s", k=128)[:, :, a0:s0 + S])

            rps, zps, xpn, ghn = [], [], [], []
            for i in range(4):
                rps.append(rzps.tile([128, 512], F32, tag="rz", name=f"rps{i}"))
                zps.append(rzps.tile([128, 512], F32, tag="rz", name=f"zps{i}"))
                xpn.append(xpps.tile([128, 512], F32, tag="xp", name=f"xpn{i}"))
                if not first:
                    ghn.append(ghps.tile([128, 512], F32, tag="gh",
                                         name=f"ghn{i}"))
            wz, wh, wr = wzn[d], wh8[d], wr8[d]
            for i in range(4):
                c0, c1 = i * 128, (i + 1) * 128
                # z and n input projections (bf16)
                for k in range(4):
                    nc.tensor.matmul(zps[i][:, :w], wz[:, k, c0:c1],
                                     xt[:, k, :w], start=k == 0,
                                     stop=first and not masked and k == 3)
                    nc.tensor.matmul(xpn[i][:, :w], wz[:, k, H + c0:H + c1],
                                     xt[:, k, :w], start=k == 0, stop=False)
                # r input projection (fp8)
                if use_dr:
                    for kp in (0, 2):
                        nc.tensor.matmul(rps[i][:, :w], wr[:, kp:kp + 2, c0:c1],
                                         x8[:, kp:kp + 2, :w], start=kp == 0,
                                         stop=first and kp == 2, perf_mode=DRM)
                else:
                    for k in range(4):
                        nc.tensor.matmul(rps[i][:, :w], wr[:, k, c0:c1],
                                         x8[:, k, :w], start=k == 0,
                                         stop=first and k == 3)
                # hidden projections (fp8), accumulate into same groups
                if not first:
                    h8 = st.h8
                    if use_dr:
                        for kp in (0, 2):
                            hk = h8[:, kp:kp + 2, so:]
                            last = kp == 2
                            nc.tensor.matmul(rps[i][:, :w], wh[:, kp:kp + 2, c0:c1],
                                             hk, start=False, stop=last,
                                             perf_mode=DRM)
                            nc.tensor.matmul(zps[i][:, :w],
                                             wh[:, kp:kp + 2, H + c0:H + c1],
                                             hk, start=False,
                                             stop=last and not masked,
                                             perf_mode=DRM)
                            nc.tensor.matmul(ghn[i][:, :w],
                                             wh[:, kp:kp + 2, 2 * H + c0:2 * H + c1],
                                             hk, start=kp == 0, stop=last,
                                             perf_mode=DRM)
                    else:
                        for k in range(4):
                            hk = st.h8[:, k, so:]
                            last = k == 3
                            nc.tensor.matmul(rps[i][:, :w], wh[:, k, c0:c1],
                                             hk, start=False, stop=last)
                            nc.tensor.matmul(zps[i][:, :w], wh[:, k, H + c0:H + c1],
                                             hk, start=False,
                                             stop=last and not masked)
                            nc.tensor.matmul(ghn[i][:, :w],
                                             wh[:, k, 2 * H + c0:2 * H + c1],
                                             hk, start=k == 0, stop=last)
                if masked:
                    row = 8 - nsteps + j
                    msl = mrows[d][0:1, row * Bc + a0:row * Bc + s0 + S]
                    nc.tensor.matmul(zps[i][:, :w], m640[0:1, :], msl,
                                     start=False, stop=True)

            r_g = gpool.tile([128, 4, S], BF16, tag="g", name="r_g")
            z_g = gpool.tile([128, 4, S], BF16, tag="g", name="z_g")
            tt = gpool.tile([128, 4, S], BF16, tag="g", name="tt")
            for i in range(4):
                nc.scalar.activation(r_g[:, i, :w], rps[i][:, :w], ACT.Sigmoid,
                                     bias=bt[:, bb + i:bb + i + 1],
                                     scale=1.0 / SCALE)
                nc.scalar.activation(z_g[:, i, :w], zps[i][:, :w], ACT.Sigmoid,
                                     bias=bt[:, bb + 4 + i:bb + 5 + i],
                                     scale=1.0 / SCALE)
                if first:
                    nc.vector.tensor_scalar_mul(tt[:, i, :w], r_g[:, i, :w],
                                                bt[:, bb + 8 + i:bb + 9 + i])
                else:
                    nc.vector.scalar_tensor_tensor(
                        tt[:, i, :w], ghn[i][:, :w],
                        bt[:, bb + 8 + i:bb + 9 + i], r_g[:, i, :w],
                        op0=ALU.add, op1=ALU.mult)
            return w, so, bb, xpn, z_g, tt

        def phase_b(d, t, steps, j, st, pa):
            """id-matmul, n ACT, h update, fp8 cast."""
            w, so, bb, xpn, z_g, tt = pa
            nsteps = len(steps)
            first = j == 0
            last = j == nsteps - 1
            n_g = gpool.tile([128, 4, S], BF16, tag="g", name="n_g")
            for i in range(4):
                nc.tensor.matmul(xpn[i][:, :w], ident[:, :], tt[:, i, :w],
                                 start=False, stop=True)
                nc.scalar.activation(n_g[:, i, :w], xpn[i][:, :w], ACT.Tanh,
                                     bias=bt[:, bb + 12 + i:bb + 13 + i],
                                     scale=1.0 / SCALE)
            h_next = (hpool if not last else hpool).tile(
                [128, 4, S], BF16, tag="hf" if last else "h", name="h")
            if not last:
                nw = steps[j + 1][0]
                nso = S - nw
                if nso < so:
                    nc.gpsimd.memset(h_next[:, :, nso:so], 0.0)
            e = gpool.tile([128, 4, S], BF16, tag="g", name="e")
            dd = None
            if not first:
                dd = gpool.tile([128, 4, S], BF16, tag="g", name="dd")
            for hh in (0, 2):
                ns = n_g[:, hh:hh + 2, :w]
                zs = z_g[:, hh:hh + 2, :w]
                es = e[:, hh:hh + 2, :w]
                ho = h_next[:, hh:hh + 2, so:]
                if first:
                    nc.vector.tensor_mul(es, zs, ns)
                    nc.vector.tensor_sub(ho, ns, es)
                else:
                    ds = dd[:, hh:hh + 2, :w]
                    nc.vector.tensor_sub(ds, st.h[:, hh:hh + 2, so:], ns)
                    nc.vector.tensor_mul(es, zs, ds)
                    nc.vector.tensor_add(ho, ns, es)
                if not last:
                    h8n = st.h8_next
                    nc.scalar.activation(h8n[:, hh:hh + 2, nso:],
                                         h_next[:, hh:hh + 2, nso:],
                                         ACT.Copy, scale=1.0)
            st.h = h_next
            st.so = so
            if not last:
                st.h8 = st.h8_next

        def emit_dir(d, states):
            stepss = [sched[t][0 if d == 'f' else 1] for t in range(ntiles)]
            mx = max(len(s) for s in stepss)
            for k in range(mx, 0, -1):
                pas = {}
                for t in range(ntiles):
                    if len(stepss[t]) >= k:
                        j = len(stepss[t]) - k
                        st = states[t]
                        if k > 1:  # next step exists: pre-alloc its h8 tile
                            st.h8_next = h8pool.tile([128, 4, S], F8, tag="h8",
                                                     name="h8")
                        pas[t] = phase_a(d, t, stepss[t], j, st)
                for t in range(ntiles):
                    if t in pas:
                        j = len(stepss[t]) - k
                        phase_b(d, t, stepss[t], j, states[t], pas[t])

        states_f = [St() for _ in range(ntiles)]
        emit_dir('f', states_f)
        states_b = [St() for _ in range(ntiles)]
        emit_dir('b', states_b)

        def emit_mlp(t, hf, hb):
            hid = gpool.tile([128, 4, S], BF16, tag="g", name="hid")
            for i in range(4):
                ps = xpps.tile([128, 512], F32, tag="xp", name="mps")
                for k in range(8):
                    src = hf if k < 4 else hb
                    nc.tensor.matmul(ps[:], w1[:, k, i * 128:(i + 1) * 128],
                                     src[:, k % 4, :], start=k == 0, stop=k == 7)
                nc.scalar.activation(hid[:, i, :], ps[:], ACT.Relu,
                                     bias=bt[:, 32 + i:33 + i], scale=1.0 / SCALE)
            onats = [opool.tile([128, H], F32, tag="o", name=f"onat{g}")
                     for g in range(4)]
            ob = gpool.tile([128, 4, S], BF16, tag="g", name="ob")
            for i in range(4):
                ps = xpps.tile([128, 512], F32, tag="xp", name="ops")
                for k in range(4):
                    nc.tensor.matmul(ps[:], w2[:, k, i * 128:(i + 1) * 128],
                                     hid[:, k, :], start=k == 0, stop=k == 3)
                nc.scalar.activation(ob[:, i, :], ps[:], ACT.Identity,
                                     bias=bt[:, 36 + i:37 + i], scale=1.0 / SCALE)
                for g in range(4):
                    tp = ghps.tile([128, 128], BF16, tag="gh", name="tp")
                    nc.tensor.transpose(tp[:], ob[:, i, g * 128:(g + 1) * 128],
                                        ident[:])
                    nc.vector.tensor_copy(onats[g][:, i * 128:(i + 1) * 128],
                                          tp[:])
            for g in range(4):
                r0 = t * S + g * 128
                nc.sync.dma_start(y_d[r0:r0 + 128, :], onats[g][:])

        for t in range(ntiles):
            emit_mlp(t, states_f[t].h, states_b[t].h)

    nc.compile()
    return nc


def kernel(padded_window, window_len, Wih_f, Whh_f, bih_f, bhh_f,
           Wih_b, Whh_b, bih_b, bhh_b, W1, b1, W2, b2):
    wl = np.asarray(window_len)
    lf = (wl - 1) // 2 + 1
    lb = wl // 2 + 1
    order = np.argsort(wl, kind="stable")

    Bc = B // NCORES
    ntiles = Bc // S
    lf_pc = lf[order].reshape(-1, NCORES)
    lb_pc = lb[order].reshape(-1, NCORES)

    def dir_steps(lens_pc, t):
        seg = lens_pc[t * S:(t + 1) * S]  # [S, NCORES]
        n = int(seg.max())
        steps = []
        for j in range(n):
            need = n - j
            cnt = (seg >= need).sum(axis=0)
            w = int(min(S, max(32, -(-int(cnt.max()) // 32) * 32)))
            masked = bool(cnt.min() < w)
            steps.append((w, masked))
        return tuple(steps)

    sched = tuple((dir_steps(lf_pc, t), dir_steps(lb_pc, t))
                  for t in range(ntiles))

    if sched not in _PROGRAM_CACHE:
        _PROGRAM_CACHE[sched] = _build_program(sched)
    nc = _PROGRAM_CACHE[sched]

    f32, bf16, f8 = np.float32, ml_dtypes.bfloat16, ml_dtypes.float8_e4m3
    sc = np.float32(SCALE)

    def q8(a):
        return np.asarray(a, f32).astype(f8)

    wznf = np.ascontiguousarray(
        (sc * np.concatenate([Wih_f[H:2 * H], Wih_f[2 * H:]], 0).T)).astype(bf16)
    wznb = np.ascontiguousarray(
        (sc * np.concatenate([Wih_b[H:2 * H], Wih_b[2 * H:]], 0).T)).astype(bf16)
    wr8f = q8(sc * np.ascontiguousarray(Wih_f[:H].T))
    wr8b = q8(sc * np.ascontiguousarray(Wih_b[:H].T))
    wh8f = q8(sc * np.ascontiguousarray(Whh_f.T))
    wh8b = q8(sc * np.ascontiguousarray(Whh_b.T))
    w1 = (sc * np.ascontiguousarray(W1.T)).astype(bf16)
    w2 = (sc * np.ascontiguousarray(W2.T)).astype(bf16)

    def chunks(v):  # [512] -> [4, 128]
        return np.asarray(v, f32).reshape(4, 128)

    bias = np.concatenate([
        chunks((bih_f + bhh_f)[:H]), chunks((bih_f + bhh_f)[H:2 * H]),
        chunks(sc * bhh_f[2 * H:]), chunks(bih_f[2 * H:]),
        chunks((bih_b + bhh_b)[:H]), chunks((bih_b + bhh_b)[H:2 * H]),
        chunks(sc * bhh_b[2 * H:]), chunks(bih_b[2 * H:]),
        chunks(b1), chunks(b2),
    ], 0)  # [40, 128]

    pw = np.asarray(padded_window, f32)
    in_maps = []
    p8 = np.arange(8)
    for c in range(NCORES):
        idx = order[c::NCORES]
        xT = np.ascontiguousarray(pw[idx].transpose(1, 2, 0))  # [15, 512, Bc]
        mzf = (p8[:, None] < (8 - lf[idx])[None, :]).astype(bf16).reshape(1, -1)
        mzb = (p8[:, None] < (8 - lb[idx])[None, :]).astype(bf16).reshape(1, -1)
        in_maps.append({
            "xb": xT.astype(bf16), "x8": xT.astype(f8),
            "wznf": wznf, "wznb": wznb, "wr8f": wr8f, "wr8b": wr8b,
            "wh8f": wh8f, "wh8b": wh8b, "w1": w1, "w2": w2,
            "bias": bias, "maskzf": mzf, "maskzb": mzb,
        })

    trace = bool(os.environ.get("GRU_TRACE"))
    kw = {}
    if os.environ.get("GRU_TMPDIR"):
        kw["tmpdir"] = os.environ["GRU_TMPDIR"]
    res = run_bass_kernel_spmd(nc, in_maps, core_ids=list(range(NCORES)),
                               trace=trace, **kw)
    global LAST_RESULT
    LAST_RESULT = res
    out = np.empty((B, H), f32)
    for c in range(NCORES):
        out[order[c::NCORES]] = res.results[c]["y"]
    return out
